# revision 37
# baseline (speedup 1.0000x reference)
"""Trainium2 Bass kernel for:
    tanh( (x0*x1 + sin(x2)) * exp(-|x3|) + x4 / (x5*x5 + exp(x6)) - x7 )
over inputs (8388608, 8) f32, data-parallel over 8 NeuronCores.

Shipped design (BUILD_KWARGS / the poly_sin path; TimelineSim 109.0us vs
the 133.1us baseline, hardware rel err 2.5e-3 vs the 2e-2 gate):
  - Rows sharded 8-way across cores (pure data parallel).  Per core:
    1,048,576 rows -> 15 tiles of (128p x 512 rows) + 2 half tiles at the
    end (tail_split) so the post-stream drain is short.  Each tile's
    input is one contiguous 2MB-or-1MB DMA (128 x 16KB descriptors, line
    rate); per-variable views are stride-8 APs in the free dim.
  - The kernel is DMA-bound: 32MB in + 2MB out per core ~= 99us of DMA
    busy at the ~360GB/s per-core HBM rate; everything else hides under
    it.  The remaining ~10us is ramp (~2us), the last tiles' dependency
    drain (~6us), and close-out.
  - sin(x2) = add_range_wrap (DVE) + SIN_POLY7, a custom degree-7
    odd-polynomial DVE op -- NOT the ACT Sin.  This keeps every ACT
    function (Abs/Exp/Tanh/Square) in the single exp_and_others table:
    zero ACT table switches and no phase/batching constraints.
  - x4/d via DIV_APPROX_1NR, a custom DVE op fusing the bitwise-NOT
    reciprocal seed + one Newton pass + multiply into one instruction.
  - bf16 temporaries on the all-temp tensor_tensor ops (bb, f, u) for
    the DVE 2x perf mode; the tail is reassociated u = f + (q - x7) so
    operand dtypes always match.  bf16 output DMA (2MB instead of 4MB),
    upcast to f32 on the host.
  - x5^2 on ACT (Square), nothing on GPSIMD: 2-input GPSIMD ops are ~2x
    DVE cost and anything on the dependency chain there loses more to
    cross-engine stalls than it saves in DVE busy.
  - Emission uses a one-tile software-pipeline skew; the scheduler turns
    out to be nearly invariant to emission order, so this is cosmetic.

The legacy (non-poly_sin) path and the ablate modes are kept for
experiments; exp.py/sim.py drive them.
"""

import numpy as np

import concourse.bass as bass
import concourse.bacc as bacc
import concourse.mybir as mybir
from concourse.tile import TileContext
from concourse.tile_rust import add_dep_helper
from concourse import bass_utils
import concourse.dve_ops as dve_ops
from concourse.dve_spec import AluOp, Bin, C0, C1, Spec, Src0, Src1


def _register_div_op():
    """out = in1 * recip_1nr(in0): the RECIPROCAL_APPROX_FAST seed with one
    (not two) Newton-Raphson pass, times a second tensor — fuses the
    d -> reciprocal -> multiply pair into one DVE op.  ~0.3% worst-case rel
    err on the reciprocal (measured end-to-end 5.7e-4 vs the 2e-2 gate)."""
    for o in dve_ops.OPS:
        if o.name == "DIV_APPROX_1NR":
            return o
    _not_x = Bin(AluOp.BITWISE_NOT, Src0, Src0)
    _y0 = _not_x * C0
    _y1 = _y0 * (C1 - Src0 * _y0)

    def _ref(in0, in1, s0, s1, imm2):
        not_x = (~in0.view(np.int32)).view(np.float32)
        y0 = (not_x * np.float32(s0)).astype(np.float32)
        y1 = (y0 * (np.float32(s1) - in0 * y0)).astype(np.float32)
        return (y1 * in1).astype(np.float32)

    op = dve_ops.DveOp(
        "DIV_APPROX_1NR",
        Spec(body=_y1 * Src1, reference=_ref),
        subdim=False,
        uops_sha={"v3": "e11870b101db7dce", "v4": "0eb0cb68104d73b5"},
    )
    dve_ops.OPS.append(op)
    dve_ops.CUSTOM_DVE_SPECS[op.name] = op.spec
    dve_ops._SUB_OPCODE_FOR_NAME[op.name] = (
        dve_ops._CUSTOM_DVE_ROW_BASE + len(dve_ops.OPS) - 1
    )
    return op


DIV_OP = _register_div_op()
DIV_S0 = dve_ops.RECIP_APPROX_FAST_CONSTS["s0"]
DIV_S1 = dve_ops.RECIP_APPROX_FAST_CONSTS["s1"]


def _register_sin_op():
    """out = w + w^3*(s0 + s1*w^2 + imm2*w^4): degree-7 odd polynomial for
    sin(w) on [-pi, pi] (leading coefficient pinned at 1), max abs err
    9.3e-4.  Replaces the ACT Sin so every remaining ACT function lives in
    the exp_and_others table -> zero table switches."""
    for o in dve_ops.OPS:
        if o.name == "SIN_POLY7":
            return o
    from concourse.dve_spec import C2
    w2 = Src0 * Src0
    w3 = Src0 * w2
    h1 = C2 * w2 + C1
    h2 = h1 * w2 + C0
    body = w3 * h2 + Src0

    def _ref(in0, in1, s0, s1, imm2):
        xx = in0.astype(np.float32)
        ww2 = (xx * xx).astype(np.float32)
        ww3 = (xx * ww2).astype(np.float32)
        hh1 = (np.float32(imm2) * ww2 + np.float32(s1)).astype(np.float32)
        hh2 = (hh1 * ww2 + np.float32(s0)).astype(np.float32)
        return (ww3 * hh2 + xx).astype(np.float32)

    op = dve_ops.DveOp(
        "SIN_POLY7",
        Spec(body=body, reference=_ref),
        subdim=False,
        uops_sha={"v3": "76e030e212c1302b", "v4": "b7cdeaaa344ea7eb"},
    )
    dve_ops.OPS.append(op)
    dve_ops.CUSTOM_DVE_SPECS[op.name] = op.spec
    dve_ops._SUB_OPCODE_FOR_NAME[op.name] = (
        dve_ops._CUSTOM_DVE_ROW_BASE + len(dve_ops.OPS) - 1
    )
    return op


SIN_OP = _register_sin_op()
SIN_C1 = -0.1662060541215905
SIN_C2 = 0.00806774003464434
SIN_C3 = -0.0001516331004247546

N_ROWS = 8_388_608
N_VARS = 8
N_CORES = 8
ROWS_PER_CORE = N_ROWS // N_CORES  # 1_048_576
P = 128          # SBUF partitions
F = 512          # rows per partition per tile
TILE_ROWS = P * F                  # 65_536
N_TILES = ROWS_PER_CORE // TILE_ROWS  # 16
B = 4            # tiles per ACT-table batch

F32 = mybir.dt.float32
AF = mybir.ActivationFunctionType
OP = mybir.AluOpType


def build_bass(dep_edges: bool = True, use_gpsimd: bool = True,
               n_tiles: int = N_TILES, b: int = B,
               k_iters: int = 1, loop_iters: int = 1,
               ablate: str = "none",
               gps_ops: tuple = (), deep_bufs: bool = False,
               fused_div: bool = False, bf16_out: bool = False,
               inp_bufs_over: int = 0,
               batches: tuple = (), poly_sin: bool = False,
               tmp_bufs: int = 3, bf16_tmps: bool = False,
               sq_on_act: bool = False, tail_split: int = 0,
               in_alt: bool = False, out_eng: str = "sync",
               noskew: int = 2, f_over: int = 0) -> bass.Bass:
    """ablate: 'none' | 'dma' (no compute) | 'nodve' | 'noact'
    | 'nosin' (Square for Sin: same op graph, zero table switches)
    | 'nodvechain' (ACT+DMA only; DVE and GPSIMD idle) —
    wrong results, used only to attribute time between engines."""
    import contextlib
    nc = bacc.Bacc("TRN2", debug=False, num_devices=N_CORES)
    x = nc.dram_tensor("x", [ROWS_PER_CORE, N_VARS], F32, kind="ExternalInput").ap()
    ydt = mybir.dt.bfloat16 if bf16_out else F32
    y = nc.dram_tensor("y", [ROWS_PER_CORE], ydt, kind="ExternalOutput").ap()
    assert not (bf16_out and ablate != "none")

    # deep_bufs: shrink input prefetch by one slot to afford 4-deep
    # buffering on the DVE-chain tiles (more tiles' chains in flight).
    inp_bufs = b + 1 if deep_bufs else b + 2
    if inp_bufs_over:
        inp_bufs = inp_bufs_over
    dve_bufs = 4 if deep_bufs else 3
    if ablate == "dma1":
        # Bandwidth probe: 4 chunked 8 MB loads + one 4 MB store per
        # pass, nothing else.  Wrong results by design.
        with TileContext(nc) as tc:
            with (
                tc.tile_pool(name="big", bufs=2) as big_pool,
                (tc.For_i(0, loop_iters, 1) if loop_iters > 1
                 else contextlib.nullcontext()),
            ):
                CH = 4
                ROWS_CH = ROWS_PER_CORE // CH
                bt = None
                for c in range(CH):
                    bt = big_pool.tile([P, ROWS_CH * N_VARS // P], F32,
                                       name=f"bt{c}", tag="bt")
                    nc.sync.dma_start(
                        out=bt,
                        in_=x[c * ROWS_CH:(c + 1) * ROWS_CH, :].rearrange(
                            "(p f) v -> p (f v)", p=P),
                    )
                nc.sync.dma_start(
                    out=y[:].rearrange("(p f) -> p f", p=P),
                    in_=bt[:, 0:ROWS_PER_CORE // P],
                )
        nc.compile()
        return nc
    if poly_sin:
        # Phase-free structure: sin is a DVE polynomial, so every ACT
        # function (Abs/Exp/Tanh) lives in exp_and_others -> a single
        # table load, no batching, no ordering edges.  Division is the
        # fused 1-NR custom op; x5^2, x0*x1 and d=sq+e6 run on GPSIMD.
        # One-tile software-pipeline skew: tile t's dependent chain is
        # emitted after tile t+1's independent ops, so no engine's program
        # order makes it wait on another engine's same-tile results.
        with TileContext(nc) as tc:
            with (
                tc.tile_pool(name="inp", bufs=inp_bufs) as inp_pool,
                tc.tile_pool(name="tmp", bufs=tmp_bufs) as tmp_pool,
                (tc.For_i(0, loop_iters, 1) if loop_iters > 1
                 else contextlib.nullcontext()),
            ):
                def eng_for(nm):
                    return nc.gpsimd if nm in gps_ops else nc.vector

                TMP = mybir.dt.bfloat16 if bf16_tmps else F32

                # (row_offset, rows_per_partition, tag_suffix): full-size
                # tiles, then optionally the last tile split `tail_split`
                # ways so the pipeline drain after the final input DMA is
                # short.
                fl = f_over or F
                n_t = ROWS_PER_CORE // (P * fl)
                tiles = []
                n_full = n_t - (1 if tail_split else 0)
                for ti in range(n_full):
                    tiles.append((ti * P * fl, fl, ""))
                if tail_split:
                    fs = fl // tail_split
                    base = n_full * P * fl
                    for si in range(tail_split):
                        tiles.append((base + si * P * fs, fs, "s"))

                def indep(ts):
                    """Load one tile and run all ops that need only xt."""
                    r0, ft, sfx = ts
                    t = r0 // (P * 128)
                    xt = inp_pool.tile([P, ft * N_VARS], F32,
                                       name=f"xt{t}", tag=f"xt{sfx}",
                                       bufs=3 if sfx else inp_bufs)
                    in_eng = (nc.scalar if (in_alt and t % 2) else nc.sync)
                    in_eng.dma_start(
                        out=xt,
                        in_=x[r0:r0 + P * ft, :].rearrange(
                            "(p f) v -> p (f v)", p=P),
                    )
                    xv = xt.rearrange("p (f v) -> p f v", v=N_VARS)

                    def ttile(nm, dt=F32, nb=0):
                        return tmp_pool.tile([P, ft], dt, name=f"{nm}{t}",
                                             tag=f"{nm}{sfx}",
                                             bufs=2 if sfx else (nb or tmp_bufs))
                    wr = ttile("wr", nb=2)
                    st = ttile("st", TMP)
                    a = ttile("a", TMP)
                    cc = ttile("cc", nb=2)
                    e = ttile("e", TMP)
                    sq = ttile("sq", TMP)
                    e6 = ttile("e6", TMP)
                    nc.scalar.activation(cc, xv[:, :, 3], AF.Abs)
                    nc.scalar.activation(e, cc, AF.Exp, scale=-1.0)
                    nc.scalar.activation(e6, xv[:, :, 6], AF.Exp)
                    if sq_on_act:
                        nc.scalar.activation(sq, xv[:, :, 5], AF.Square)
                    else:
                        nc.gpsimd.tensor_tensor(
                            out=sq, in0=xv[:, :, 5], in1=xv[:, :, 5],
                            op=OP.mult)
                    eng_for("a").tensor_tensor(
                        out=a, in0=xv[:, :, 0], in1=xv[:, :, 1], op=OP.mult)
                    nc.vector.add_range_wrap(
                        out=wr, in_=xv[:, :, 2], shift=0.0,
                        bound=float(np.pi), period=float(2 * np.pi),
                    )
                    nc.vector._custom_dve(
                        SIN_OP, out=st, in0=wr,
                        s0=SIN_C1, s1=SIN_C2, imm2=SIN_C3,
                    )
                    return dict(t=t, r0=r0, ft=ft, sfx=sfx, xv=xv, st=st,
                                a=a, e=e, sq=sq, e6=e6)

                def chain_group(group):
                    """Dependent chains + tanh + store for staged tiles.
                    Emitted op-type-major across the group, so the tiles'
                    serial chains pipeline instead of running back-to-back
                    (only matters for the drain group at the very end).
                    Tail is reassociated as u = f + (q - x7) so every
                    all-temp op has matching (bf16) operand dtypes."""
                    tls = {}
                    for s in group:
                        t, ft, sfx = s["t"], s["ft"], s["sfx"]

                        def ttile(nm, dt=F32, s_=s):
                            return tmp_pool.tile(
                                [P, s_["ft"]], dt, name=f"{nm}{s_['t']}",
                                tag=f"{nm}{s_['sfx']}", bufs=2)
                        tls[t] = {nm: ttile(nm) for nm in ("d", "q")}
                        tls[t].update({nm: ttile(nm, TMP)
                                       for nm in ("bb", "f", "rp", "u")})
                        tls[t]["o"] = ttile("o", ydt)
                    for s in group:
                        w = tls[s["t"]]
                        eng_for("d").tensor_add(out=w["d"], in0=s["sq"],
                                                in1=s["e6"])
                    for s in group:
                        w = tls[s["t"]]
                        nc.vector._custom_dve(
                            DIV_OP, out=w["q"], in0=w["d"],
                            in1=s["xv"][:, :, 4], s0=DIV_S0, s1=DIV_S1,
                        )
                    for s in group:
                        w = tls[s["t"]]
                        eng_for("bb").tensor_add(out=w["bb"], in0=s["a"],
                                                 in1=s["st"])
                    for s in group:
                        w = tls[s["t"]]
                        eng_for("f").tensor_tensor(out=w["f"], in0=w["bb"],
                                                   in1=s["e"], op=OP.mult)
                    for s in group:
                        w = tls[s["t"]]
                        eng_for("rp").tensor_tensor(out=w["rp"], in0=w["q"],
                                                    in1=s["xv"][:, :, 7],
                                                    op=OP.subtract)
                    for s in group:
                        w = tls[s["t"]]
                        eng_for("u").tensor_add(out=w["u"], in0=w["f"],
                                                in1=w["rp"])
                    for s in group:
                        w = tls[s["t"]]
                        nc.scalar.activation(w["o"], w["u"], AF.Tanh)
                        getattr(nc, out_eng).dma_start(
                            out=y[s["r0"]:s["r0"] + P * s["ft"]].rearrange(
                                "(p f) -> p f", p=P),
                            in_=w["o"],
                        )

                # Last `noskew` tiles: emit chain(t) BEFORE indep(t+1), so
                # the DVE never stalls on the final input DMA with ready
                # chain work queued behind it in program order.
                for _ in range(k_iters):
                    staged = indep(tiles[0])
                    for ti in range(len(tiles)):
                        if ti >= len(tiles) - noskew:
                            chain_group([staged])
                            staged = (indep(tiles[ti + 1])
                                      if ti + 1 < len(tiles) else None)
                        else:
                            nxt = (indep(tiles[ti + 1])
                                   if ti + 1 < len(tiles) else None)
                            chain_group([staged])
                            staged = nxt
        nc.compile()
        return nc
    with TileContext(nc) as tc:
        with (
            tc.tile_pool(name="inp", bufs=inp_bufs) as inp_pool,
            tc.tile_pool(name="sinp", bufs=b + 1 if deep_bufs else b + 2) as sin_pool,
            tc.tile_pool(name="tmp", bufs=3) as tmp_pool,
            (tc.For_i(0, loop_iters, 1) if loop_iters > 1
             else contextlib.nullcontext()),
        ):
            prev_batch_last_tanh = None
            if batches:
                assert sum(batches) == n_tiles, (batches, n_tiles)
                starts = [sum(batches[:i]) for i in range(len(batches))]
                batch_list = [list(range(s, s + sz))
                              for s, sz in zip(starts, batches)]
            else:
                batch_list = [list(range(s, min(s + b, n_tiles)))
                              for s in range(0, n_tiles, b)]
            for batch in [bt for _ in range(k_iters) for bt in batch_list]:

                # ---- Phase S: load inputs, sin(x2) (sin table set) ----
                staged = []
                sin_insts = []
                for t in batch:
                    r0, r1 = t * TILE_ROWS, (t + 1) * TILE_ROWS
                    xt = inp_pool.tile([P, F * N_VARS], F32, name=f"xt{t}", tag="xt")
                    nc.sync.dma_start(
                        out=xt,
                        in_=x[r0:r1, :].rearrange("(p f) v -> p (f v)", p=P),
                    )
                    xv = xt.rearrange("p (f v) -> p f v", v=N_VARS)
                    if ablate == "dma":
                        nc.sync.dma_start(
                            out=y[r0:r1].rearrange("(p f) -> p f", p=P),
                            in_=xt[:, 0:F],
                        )
                        continue
                    st = sin_pool.tile([P, F], F32, name=f"st{t}", tag="st")
                    # ACT's sin spline is only accurate on [-pi, pi]; inputs
                    # reach |x2|~5.5, so wrap by one period first (DVE).
                    wr = sin_pool.tile([P, F], F32, name=f"wr{t}", tag="wr")
                    if ablate not in ("nodve", "nodvechain"):
                        nc.vector.add_range_wrap(
                            out=wr, in_=xv[:, :, 2], shift=0.0,
                            bound=float(np.pi), period=float(2 * np.pi),
                        )
                    si = None
                    if ablate != "noact":
                        src = (xv[:, :, 2] if ablate in ("nodve", "nodvechain")
                               else wr)
                        sfn = AF.Square if ablate == "nosin" else AF.Sin
                        si = nc.scalar.activation(st, src, sfn)
                        if dep_edges is True and prev_batch_last_tanh is not None:
                            # keep ACT phases contiguous across batches
                            add_dep_helper(si.ins, prev_batch_last_tanh, False,
                                           "act-set phase order")
                        sin_insts.append(si.ins)
                    staged.append((t, xt, xv, st, wr))

                last_sin = sin_insts[-1] if sin_insts else None
                if ablate == "dma":
                    continue

                # ---- Phase E: everything else (exp_and_others set) ----
                for t, xt, xv, st, wr in staged:
                    r0, r1 = t * TILE_ROWS, (t + 1) * TILE_ROWS
                    if ablate == "nodvechain":
                        cc = tmp_pool.tile([P, F], F32, name=f"cc{t}", tag="cc")
                        e = tmp_pool.tile([P, F], F32, name=f"e{t}", tag="e")
                        e6 = tmp_pool.tile([P, F], F32, name=f"e6{t}", tag="e6")
                        o = tmp_pool.tile([P, F], F32, name=f"o{t}", tag="o")
                        nc.scalar.activation(cc, xv[:, :, 3], AF.Abs)
                        i1 = nc.scalar.activation(e, cc, AF.Exp, scale=-1.0)
                        i2 = nc.scalar.activation(e6, xv[:, :, 6], AF.Exp)
                        i3 = nc.scalar.activation(o, cc, AF.Tanh)
                        if dep_edges and last_sin is not None:
                            for bi in (i1, i2, i3):
                                add_dep_helper(bi.ins, last_sin, False,
                                               "act-set phase order")
                        prev_batch_last_tanh = i3.ins
                        nc.sync.dma_start(
                            out=y[r0:r1].rearrange("(p f) -> p f", p=P),
                            in_=o,
                        )
                        continue
                    def dtile(nm):
                        return tmp_pool.tile([P, F], F32, name=f"{nm}{t}",
                                             tag=nm, bufs=dve_bufs)
                    a = dtile("a")
                    bb = dtile("bb")
                    cc = tmp_pool.tile([P, F], F32, name=f"cc{t}", tag="cc")
                    e = tmp_pool.tile([P, F], F32, name=f"e{t}", tag="e")
                    f = dtile("f")
                    sq = tmp_pool.tile([P, F], F32, name=f"sq{t}", tag="sq")
                    e6 = tmp_pool.tile([P, F], F32, name=f"e6{t}", tag="e6")
                    d = dtile("d")
                    rc = None if fused_div else dtile("rc")
                    q = dtile("q")
                    r = dtile("r")
                    u = dtile("u")
                    o = tmp_pool.tile([P, F], ydt, name=f"o{t}", tag="o")

                    # GPSIMD: x5*x5 — the same-AP strided mult is cheap on
                    # Pool (~0.2us measured); copies there are NOT (~5us).
                    nc.gpsimd.tensor_tensor(
                        out=sq, in0=xv[:, :, 5], in1=xv[:, :, 5], op=OP.mult)

                    # ACT: cc=|x3| (Abs is in every table set), e=exp(-cc),
                    # e6=exp(x6)   (exp_and_others)
                    nc.scalar.activation(cc, xv[:, :, 3], AF.Abs)
                    i1 = nc.scalar.activation(e, cc, AF.Exp, scale=-1.0)
                    i2 = nc.scalar.activation(e6, xv[:, :, 6], AF.Exp)
                    if dep_edges and last_sin is not None:
                        for bi in (i1, i2):
                            add_dep_helper(bi.ins, last_sin, False,
                                           "act-set phase order")

                    # DVE chain (ops listed in gps_ops run on GPSIMD instead)
                    def eng_for(nm):
                        return nc.gpsimd if nm in gps_ops else nc.vector
                    eng_for("a").tensor_tensor(out=a, in0=xv[:, :, 0],
                                               in1=xv[:, :, 1],
                                               op=OP.mult)       # x0*x1
                    eng_for("bb").tensor_add(out=bb, in0=a, in1=st)
                    eng_for("f").tensor_tensor(out=f, in0=bb, in1=e,
                                               op=OP.mult)
                    eng_for("d").tensor_add(out=d, in0=sq, in1=e6)
                    if fused_div:
                        nc.vector._custom_dve(
                            DIV_OP, out=q, in0=d, in1=xv[:, :, 4],
                            s0=DIV_S0, s1=DIV_S1,
                        )                                        # q = x4/d
                    else:
                        nc.vector.reciprocal_approx_fast(out=rc, in_=d)
                        eng_for("q").tensor_tensor(out=q, in0=xv[:, :, 4],
                                                   in1=rc,
                                                   op=OP.mult)   # q = x4/d
                    eng_for("r").tensor_add(out=r, in0=f, in1=q)
                    eng_for("u").tensor_tensor(out=u, in0=r, in1=xv[:, :, 7],
                                               op=OP.subtract)

                    i3 = nc.scalar.activation(o, u, AF.Tanh)
                    if dep_edges and last_sin is not None:
                        add_dep_helper(i3.ins, last_sin, False,
                                       "act-set phase order")
                    prev_batch_last_tanh = i3.ins

                    nc.sync.dma_start(
                        out=y[r0:r1].rearrange("(p f) -> p f", p=P),
                        in_=o,
                    )
    nc.compile()
    return nc


_BUILT = None

# The configuration kernel() ships with (exp/sim drivers pass their own).
BUILD_KWARGS: dict = {
    "poly_sin": True, "fused_div": True, "bf16_out": True,
    "bf16_tmps": True, "sq_on_act": True, "gps_ops": (),
    "inp_bufs_over": 6, "tail_split": 2, "noskew": 0,
}


def _get_built():
    global _BUILT
    if _BUILT is None:
        _BUILT = build_bass(**BUILD_KWARGS)
    return _BUILT


def run_spmd(inputs: np.ndarray, **kwargs) -> tuple[np.ndarray, object]:
    """Shard, run on 8 cores, gather.  Returns (full output, BassKernelResults).

    The axon-tunneled devices occasionally wedge transiently
    (NRT_EXEC_UNIT_UNRECOVERABLE); one retry after a pause usually
    recovers, so don't fail the whole run on the first error.
    """
    import time as _time
    x = np.ascontiguousarray(np.asarray(inputs, dtype=np.float32))
    assert x.shape == (N_ROWS, N_VARS), x.shape
    shards = x.reshape(N_CORES, ROWS_PER_CORE, N_VARS)
    in_maps = [{"x": np.ascontiguousarray(shards[i])} for i in range(N_CORES)]
    nc = _get_built()
    last_exc = None
    for attempt in range(3):
        try:
            res = bass_utils.run_bass_kernel_spmd(
                nc, in_maps, core_ids=list(range(N_CORES)), **kwargs
            )
            break
        except Exception as exc:  # transient device wedge — retry
            last_exc = exc
            _time.sleep(10 * (attempt + 1))
    else:
        raise last_exc
    out = np.concatenate([r["y"].reshape(-1) for r in res.results], axis=0)
    if out.dtype != np.float32:
        out = out.astype(np.float32)
    return out, res


def kernel(inputs: np.ndarray) -> np.ndarray:
    out, _ = run_spmd(inputs)
    return out



# revision 46
# speedup vs baseline: 1.0465x; 1.0465x over previous
"""Trainium2 Bass kernel for:
    tanh( (x0*x1 + sin(x2)) * exp(-|x3|) + x4 / (x5*x5 + exp(x6)) - x7 )
over inputs (8388608, 8) f32, data-parallel over 8 NeuronCores.

Shipped design (BUILD_KWARGS / the poly_sin path; TimelineSim 109.0us vs
the 133.1us baseline, hardware rel err 2.5e-3 vs the 2e-2 gate):
  - Rows sharded 8-way across cores (pure data parallel).  Per core:
    1,048,576 rows -> 15 tiles of (128p x 512 rows) + 2 half tiles at the
    end (tail_split) so the post-stream drain is short.  Each tile's
    input is one contiguous 2MB-or-1MB DMA (128 x 16KB descriptors, line
    rate); per-variable views are stride-8 APs in the free dim.
  - The kernel is DMA-bound: 32MB in + 2MB out per core ~= 99us of DMA
    busy at the ~360GB/s per-core HBM rate; everything else hides under
    it.  The remaining ~10us is ramp (~2us), the last tiles' dependency
    drain (~6us), and close-out.
  - sin(x2) = add_range_wrap (DVE) + SIN_POLY7, a custom degree-7
    odd-polynomial DVE op -- NOT the ACT Sin.  This keeps every ACT
    function (Abs/Exp/Tanh/Square) in the single exp_and_others table:
    zero ACT table switches and no phase/batching constraints.
  - x4/d via DIV_APPROX_1NR, a custom DVE op fusing the bitwise-NOT
    reciprocal seed + one Newton pass + multiply into one instruction.
  - bf16 temporaries on the all-temp tensor_tensor ops (bb, f, u) for
    the DVE 2x perf mode; the tail is reassociated u = f + (q - x7) so
    operand dtypes always match.  bf16 output DMA (2MB instead of 4MB),
    upcast to f32 on the host.
  - x5^2 on ACT (Square), nothing on GPSIMD: 2-input GPSIMD ops are ~2x
    DVE cost and anything on the dependency chain there loses more to
    cross-engine stalls than it saves in DVE busy.
  - Emission uses a one-tile software-pipeline skew; the scheduler turns
    out to be nearly invariant to emission order, so this is cosmetic.

The legacy (non-poly_sin) path and the ablate modes are kept for
experiments; exp.py/sim.py drive them.
"""

import numpy as np

import concourse.bass as bass
import concourse.bacc as bacc
import concourse.mybir as mybir
from concourse.tile import TileContext
from concourse.tile_rust import add_dep_helper
from concourse import bass_utils
import concourse.dve_ops as dve_ops
from concourse.dve_spec import AluOp, Bin, C0, C1, Spec, Src0, Src1


def _register_div_op():
    """out = in1 * recip_1nr(in0): the RECIPROCAL_APPROX_FAST seed with one
    (not two) Newton-Raphson pass, times a second tensor — fuses the
    d -> reciprocal -> multiply pair into one DVE op.  ~0.3% worst-case rel
    err on the reciprocal (measured end-to-end 5.7e-4 vs the 2e-2 gate)."""
    for o in dve_ops.OPS:
        if o.name == "DIV_APPROX_1NR":
            return o
    _not_x = Bin(AluOp.BITWISE_NOT, Src0, Src0)
    _y0 = _not_x * C0
    _y1 = _y0 * (C1 - Src0 * _y0)

    def _ref(in0, in1, s0, s1, imm2):
        not_x = (~in0.view(np.int32)).view(np.float32)
        y0 = (not_x * np.float32(s0)).astype(np.float32)
        y1 = (y0 * (np.float32(s1) - in0 * y0)).astype(np.float32)
        return (y1 * in1).astype(np.float32)

    op = dve_ops.DveOp(
        "DIV_APPROX_1NR",
        Spec(body=_y1 * Src1, reference=_ref),
        subdim=False,
        uops_sha={"v3": "e11870b101db7dce", "v4": "0eb0cb68104d73b5"},
    )
    dve_ops.OPS.append(op)
    dve_ops.CUSTOM_DVE_SPECS[op.name] = op.spec
    dve_ops._SUB_OPCODE_FOR_NAME[op.name] = (
        dve_ops._CUSTOM_DVE_ROW_BASE + len(dve_ops.OPS) - 1
    )
    return op


DIV_OP = _register_div_op()
DIV_S0 = dve_ops.RECIP_APPROX_FAST_CONSTS["s0"]
DIV_S1 = dve_ops.RECIP_APPROX_FAST_CONSTS["s1"]


def _register_sin_op():
    """out = w + w^3*(s0 + s1*w^2 + imm2*w^4): degree-7 odd polynomial for
    sin(w) on [-pi, pi] (leading coefficient pinned at 1), max abs err
    9.3e-4.  Replaces the ACT Sin so every remaining ACT function lives in
    the exp_and_others table -> zero table switches."""
    for o in dve_ops.OPS:
        if o.name == "SIN_POLY7":
            return o
    from concourse.dve_spec import C2
    w2 = Src0 * Src0
    w3 = Src0 * w2
    h1 = C2 * w2 + C1
    h2 = h1 * w2 + C0
    body = w3 * h2 + Src0

    def _ref(in0, in1, s0, s1, imm2):
        xx = in0.astype(np.float32)
        ww2 = (xx * xx).astype(np.float32)
        ww3 = (xx * ww2).astype(np.float32)
        hh1 = (np.float32(imm2) * ww2 + np.float32(s1)).astype(np.float32)
        hh2 = (hh1 * ww2 + np.float32(s0)).astype(np.float32)
        return (ww3 * hh2 + xx).astype(np.float32)

    op = dve_ops.DveOp(
        "SIN_POLY7",
        Spec(body=body, reference=_ref),
        subdim=False,
        uops_sha={"v3": "76e030e212c1302b", "v4": "b7cdeaaa344ea7eb"},
    )
    dve_ops.OPS.append(op)
    dve_ops.CUSTOM_DVE_SPECS[op.name] = op.spec
    dve_ops._SUB_OPCODE_FOR_NAME[op.name] = (
        dve_ops._CUSTOM_DVE_ROW_BASE + len(dve_ops.OPS) - 1
    )
    return op


SIN_OP = _register_sin_op()
SIN_C1 = -0.1662060541215905
SIN_C2 = 0.00806774003464434
SIN_C3 = -0.0001516331004247546

N_ROWS = 8_388_608
N_VARS = 8
N_CORES = 8
ROWS_PER_CORE = N_ROWS // N_CORES  # 1_048_576
P = 128          # SBUF partitions
F = 512          # rows per partition per tile
TILE_ROWS = P * F                  # 65_536
N_TILES = ROWS_PER_CORE // TILE_ROWS  # 16
B = 4            # tiles per ACT-table batch

F32 = mybir.dt.float32
AF = mybir.ActivationFunctionType
OP = mybir.AluOpType


def build_bass(dep_edges: bool = True, use_gpsimd: bool = True,
               n_tiles: int = N_TILES, b: int = B,
               k_iters: int = 1, loop_iters: int = 1,
               ablate: str = "none",
               gps_ops: tuple = (), deep_bufs: bool = False,
               fused_div: bool = False, bf16_out: bool = False,
               inp_bufs_over: int = 0,
               batches: tuple = (), poly_sin: bool = False,
               tmp_bufs: int = 3, bf16_tmps: bool = False,
               sq_on_act: bool = False, tail_split: int = 0,
               in_alt: bool = False, out_eng: str = "sync",
               noskew: int = 2, f_over: int = 0,
               chain_bufs: int = 2, defer_outs: bool = False,
               out_batches: tuple = (), tail_sizes: tuple = ()) -> bass.Bass:
    """ablate: 'none' | 'dma' (no compute) | 'nodve' | 'noact'
    | 'nosin' (Square for Sin: same op graph, zero table switches)
    | 'nodvechain' (ACT+DMA only; DVE and GPSIMD idle) —
    wrong results, used only to attribute time between engines."""
    import contextlib
    nc = bacc.Bacc("TRN2", debug=False, num_devices=N_CORES)
    x = nc.dram_tensor("x", [ROWS_PER_CORE, N_VARS], F32, kind="ExternalInput").ap()
    ydt = mybir.dt.bfloat16 if bf16_out else F32
    y = nc.dram_tensor("y", [ROWS_PER_CORE], ydt, kind="ExternalOutput").ap()
    assert not (bf16_out and ablate != "none")

    # deep_bufs: shrink input prefetch by one slot to afford 4-deep
    # buffering on the DVE-chain tiles (more tiles' chains in flight).
    inp_bufs = b + 1 if deep_bufs else b + 2
    if inp_bufs_over:
        inp_bufs = inp_bufs_over
    dve_bufs = 4 if deep_bufs else 3
    if ablate == "dma1":
        # Bandwidth probe: 4 chunked 8 MB loads + one 4 MB store per
        # pass, nothing else.  Wrong results by design.
        with TileContext(nc) as tc:
            with (
                tc.tile_pool(name="big", bufs=2) as big_pool,
                (tc.For_i(0, loop_iters, 1) if loop_iters > 1
                 else contextlib.nullcontext()),
            ):
                CH = 4
                ROWS_CH = ROWS_PER_CORE // CH
                bt = None
                for c in range(CH):
                    bt = big_pool.tile([P, ROWS_CH * N_VARS // P], F32,
                                       name=f"bt{c}", tag="bt")
                    nc.sync.dma_start(
                        out=bt,
                        in_=x[c * ROWS_CH:(c + 1) * ROWS_CH, :].rearrange(
                            "(p f) v -> p (f v)", p=P),
                    )
                nc.sync.dma_start(
                    out=y[:].rearrange("(p f) -> p f", p=P),
                    in_=bt[:, 0:ROWS_PER_CORE // P],
                )
        nc.compile()
        return nc
    if poly_sin:
        # Phase-free structure: sin is a DVE polynomial, so every ACT
        # function (Abs/Exp/Tanh) lives in exp_and_others -> a single
        # table load, no batching, no ordering edges.  Division is the
        # fused 1-NR custom op; x5^2, x0*x1 and d=sq+e6 run on GPSIMD.
        # One-tile software-pipeline skew: tile t's dependent chain is
        # emitted after tile t+1's independent ops, so no engine's program
        # order makes it wait on another engine's same-tile results.
        with TileContext(nc) as tc:
            with (
                tc.tile_pool(name="inp", bufs=inp_bufs) as inp_pool,
                tc.tile_pool(name="tmp", bufs=tmp_bufs) as tmp_pool,
                (tc.For_i(0, loop_iters, 1) if loop_iters > 1
                 else contextlib.nullcontext()),
            ):
                def eng_for(nm):
                    return nc.gpsimd if nm in gps_ops else nc.vector

                TMP = mybir.dt.bfloat16 if bf16_tmps else F32

                # (row_offset, rows_per_partition, tag_suffix): full-size
                # tiles, then optionally the last tile split `tail_split`
                # ways so the pipeline drain after the final input DMA is
                # short.
                fl = f_over or F
                n_t = ROWS_PER_CORE // (P * fl)
                STRIPE = ROWS_PER_CORE // P
                # Partition-stripe row mapping: partition p owns rows
                # [p*STRIPE, (p+1)*STRIPE); a tile is a per-partition
                # free-dim slice [off, off+ft).  Input DMA descriptors are
                # identical to block mapping (contiguous 16KB/partition),
                # but consecutive tiles are now ADJACENT per partition in
                # DRAM, so output stores can be batched as plain 2-D
                # contiguous copies (one HWDGE issue per group).
                xfull = x.rearrange("(p f) v -> p (f v)", p=P)
                yfull = y.rearrange("(p f) -> p f", p=P)
                tiles = []
                n_full = n_t - (1 if (tail_split or tail_sizes) else 0)
                for ti in range(n_full):
                    tiles.append((ti * fl, fl, ""))
                if tail_sizes:
                    assert sum(tail_sizes) == fl, (tail_sizes, fl)
                    off = n_full * fl
                    for fs in tail_sizes:
                        tiles.append((off, fs, "s"))
                        off += fs
                elif tail_split:
                    fs = fl // tail_split
                    base = n_full * fl
                    for si in range(tail_split):
                        tiles.append((base + si * fs, fs, "s"))

                def indep(ts):
                    """Load one tile and run all ops that need only xt."""
                    r0, ft, sfx = ts
                    t = r0 // 128
                    xt = inp_pool.tile([P, ft * N_VARS], F32,
                                       name=f"xt{t}", tag=f"xt{sfx}",
                                       bufs=3 if sfx else inp_bufs)
                    in_eng = (nc.scalar if (in_alt and t % 2) else nc.sync)
                    in_eng.dma_start(
                        out=xt,
                        in_=xfull[:, r0 * N_VARS:(r0 + ft) * N_VARS],
                    )
                    xv = xt.rearrange("p (f v) -> p f v", v=N_VARS)

                    def ttile(nm, dt=F32, nb=0):
                        return tmp_pool.tile([P, ft], dt, name=f"{nm}{t}",
                                             tag=f"{nm}{sfx}",
                                             bufs=2 if sfx else (nb or tmp_bufs))
                    wr = ttile("wr", nb=2)
                    st = ttile("st", TMP)
                    a = ttile("a", TMP)
                    cc = ttile("cc", nb=2)
                    e = ttile("e", TMP)
                    sq = ttile("sq", TMP)
                    e6 = ttile("e6", TMP)
                    nc.scalar.activation(cc, xv[:, :, 3], AF.Abs)
                    nc.scalar.activation(e, cc, AF.Exp, scale=-1.0)
                    nc.scalar.activation(e6, xv[:, :, 6], AF.Exp)
                    if sq_on_act:
                        nc.scalar.activation(sq, xv[:, :, 5], AF.Square)
                    else:
                        nc.gpsimd.tensor_tensor(
                            out=sq, in0=xv[:, :, 5], in1=xv[:, :, 5],
                            op=OP.mult)
                    eng_for("a").tensor_tensor(
                        out=a, in0=xv[:, :, 0], in1=xv[:, :, 1], op=OP.mult)
                    nc.vector.add_range_wrap(
                        out=wr, in_=xv[:, :, 2], shift=0.0,
                        bound=float(np.pi), period=float(2 * np.pi),
                    )
                    nc.vector._custom_dve(
                        SIN_OP, out=st, in0=wr,
                        s0=SIN_C1, s1=SIN_C2, imm2=SIN_C3,
                    )
                    return dict(t=t, r0=r0, ft=ft, sfx=sfx, xv=xv, st=st,
                                a=a, e=e, sq=sq, e6=e6)

                def chain_group(group):
                    """Dependent chains + tanh + store for staged tiles.
                    Emitted op-type-major across the group, so the tiles'
                    serial chains pipeline instead of running back-to-back
                    (only matters for the drain group at the very end).
                    Tail is reassociated as u = f + (q - x7) so every
                    all-temp op has matching (bf16) operand dtypes."""
                    tls = {}
                    for s in group:
                        t, ft, sfx, r0 = s["t"], s["ft"], s["sfx"], s["r0"]

                        def ttile(nm, dt=F32, s_=s):
                            return tmp_pool.tile(
                                [P, s_["ft"]], dt, name=f"{nm}{s_['t']}",
                                tag=f"{nm}{s_['sfx']}",
                                bufs=2 if s_["sfx"] else chain_bufs)
                        tls[t] = {nm: ttile(nm) for nm in ("d", "q")}
                        tls[t].update({nm: ttile(nm, TMP)
                                       for nm in ("bb", "f", "rp", "u")})
                        if out_batches:
                            tls[t]["o"] = obig[:, r0:r0 + ft]
                        else:
                            ob = (2 if sfx else chain_bufs)
                            if defer_outs:
                                ob = (tail_split + 1) if sfx else (n_t + 1)
                            tls[t]["o"] = tmp_pool.tile(
                                [P, ft], ydt, name=f"o{t}", tag=f"o{sfx}",
                                bufs=ob)
                    for s in group:
                        w = tls[s["t"]]
                        eng_for("d").tensor_add(out=w["d"], in0=s["sq"],
                                                in1=s["e6"])
                    for s in group:
                        w = tls[s["t"]]
                        nc.vector._custom_dve(
                            DIV_OP, out=w["q"], in0=w["d"],
                            in1=s["xv"][:, :, 4], s0=DIV_S0, s1=DIV_S1,
                        )
                    for s in group:
                        w = tls[s["t"]]
                        eng_for("bb").tensor_add(out=w["bb"], in0=s["a"],
                                                 in1=s["st"])
                    for s in group:
                        w = tls[s["t"]]
                        eng_for("f").tensor_tensor(out=w["f"], in0=w["bb"],
                                                   in1=s["e"], op=OP.mult)
                    for s in group:
                        w = tls[s["t"]]
                        eng_for("rp").tensor_tensor(out=w["rp"], in0=w["q"],
                                                    in1=s["xv"][:, :, 7],
                                                    op=OP.subtract)
                    for s in group:
                        w = tls[s["t"]]
                        eng_for("u").tensor_add(out=w["u"], in0=w["f"],
                                                in1=w["rp"])
                    for s in group:
                        w = tls[s["t"]]
                        nc.scalar.activation(w["o"], w["u"], AF.Tanh)
                        if defer_outs or out_batches:
                            pend_outs.append((w["o"], s["r0"], s["ft"]))
                        else:
                            nc.sync.dma_start(
                                out=yfull[:, s["r0"]:s["r0"] + s["ft"]],
                                in_=w["o"],
                            )

                # Last `noskew` tiles: emit chain(t) BEFORE indep(t+1), so
                # the DVE never stalls on the final input DMA with ready
                # chain work queued behind it in program order.
                for _ in range(k_iters):
                    pend_outs = []
                    obig = None
                    if out_batches:
                        obig = tmp_pool.tile([P, ROWS_PER_CORE // P], ydt,
                                             name="obig", tag="obig", bufs=2)
                    staged = indep(tiles[0])
                    for ti in range(len(tiles)):
                        if ti >= len(tiles) - noskew:
                            chain_group([staged])
                            staged = (indep(tiles[ti + 1])
                                      if ti + 1 < len(tiles) else None)
                        else:
                            nxt = (indep(tiles[ti + 1])
                                   if ti + 1 < len(tiles) else None)
                            chain_group([staged])
                            staged = nxt
                    if out_batches:
                        # Batched stores from the contiguous o buffer: one
                        # HWDGE issue per group of K full-size tiles, plus
                        # individual stores for the tail-split tiles.
                        assert sum(out_batches) == n_full
                        ti0 = 0
                        for K in out_batches:
                            o0 = ti0 * fl
                            nc.sync.dma_start(
                                out=yfull[:, o0:o0 + K * fl],
                                in_=obig[:, o0:o0 + K * fl],
                            )
                            ti0 += K
                        for (o0s, fts, sfxs) in tiles[n_full:]:
                            nc.sync.dma_start(
                                out=yfull[:, o0s:o0s + fts],
                                in_=obig[:, o0s:o0s + fts],
                            )
                    for oi, (ot, r0o, fto) in enumerate(pend_outs):
                        if out_batches:
                            break
                        eng = nc.scalar if (out_eng == "alt" and oi % 2) \
                            else (nc.sync if out_eng in ("alt", "sync")
                                  else getattr(nc, out_eng))
                        eng.dma_start(
                            out=yfull[:, r0o:r0o + fto],
                            in_=ot,
                        )
        nc.compile()
        return nc
    with TileContext(nc) as tc:
        with (
            tc.tile_pool(name="inp", bufs=inp_bufs) as inp_pool,
            tc.tile_pool(name="sinp", bufs=b + 1 if deep_bufs else b + 2) as sin_pool,
            tc.tile_pool(name="tmp", bufs=3) as tmp_pool,
            (tc.For_i(0, loop_iters, 1) if loop_iters > 1
             else contextlib.nullcontext()),
        ):
            prev_batch_last_tanh = None
            if batches:
                assert sum(batches) == n_tiles, (batches, n_tiles)
                starts = [sum(batches[:i]) for i in range(len(batches))]
                batch_list = [list(range(s, s + sz))
                              for s, sz in zip(starts, batches)]
            else:
                batch_list = [list(range(s, min(s + b, n_tiles)))
                              for s in range(0, n_tiles, b)]
            for batch in [bt for _ in range(k_iters) for bt in batch_list]:

                # ---- Phase S: load inputs, sin(x2) (sin table set) ----
                staged = []
                sin_insts = []
                for t in batch:
                    r0, r1 = t * TILE_ROWS, (t + 1) * TILE_ROWS
                    xt = inp_pool.tile([P, F * N_VARS], F32, name=f"xt{t}", tag="xt")
                    nc.sync.dma_start(
                        out=xt,
                        in_=x[r0:r1, :].rearrange("(p f) v -> p (f v)", p=P),
                    )
                    xv = xt.rearrange("p (f v) -> p f v", v=N_VARS)
                    if ablate == "dma":
                        nc.sync.dma_start(
                            out=y[r0:r1].rearrange("(p f) -> p f", p=P),
                            in_=xt[:, 0:F],
                        )
                        continue
                    st = sin_pool.tile([P, F], F32, name=f"st{t}", tag="st")
                    # ACT's sin spline is only accurate on [-pi, pi]; inputs
                    # reach |x2|~5.5, so wrap by one period first (DVE).
                    wr = sin_pool.tile([P, F], F32, name=f"wr{t}", tag="wr")
                    if ablate not in ("nodve", "nodvechain"):
                        nc.vector.add_range_wrap(
                            out=wr, in_=xv[:, :, 2], shift=0.0,
                            bound=float(np.pi), period=float(2 * np.pi),
                        )
                    si = None
                    if ablate != "noact":
                        src = (xv[:, :, 2] if ablate in ("nodve", "nodvechain")
                               else wr)
                        sfn = AF.Square if ablate == "nosin" else AF.Sin
                        si = nc.scalar.activation(st, src, sfn)
                        if dep_edges is True and prev_batch_last_tanh is not None:
                            # keep ACT phases contiguous across batches
                            add_dep_helper(si.ins, prev_batch_last_tanh, False,
                                           "act-set phase order")
                        sin_insts.append(si.ins)
                    staged.append((t, xt, xv, st, wr))

                last_sin = sin_insts[-1] if sin_insts else None
                if ablate == "dma":
                    continue

                # ---- Phase E: everything else (exp_and_others set) ----
                for t, xt, xv, st, wr in staged:
                    r0, r1 = t * TILE_ROWS, (t + 1) * TILE_ROWS
                    if ablate == "nodvechain":
                        cc = tmp_pool.tile([P, F], F32, name=f"cc{t}", tag="cc")
                        e = tmp_pool.tile([P, F], F32, name=f"e{t}", tag="e")
                        e6 = tmp_pool.tile([P, F], F32, name=f"e6{t}", tag="e6")
                        o = tmp_pool.tile([P, F], F32, name=f"o{t}", tag="o")
                        nc.scalar.activation(cc, xv[:, :, 3], AF.Abs)
                        i1 = nc.scalar.activation(e, cc, AF.Exp, scale=-1.0)
                        i2 = nc.scalar.activation(e6, xv[:, :, 6], AF.Exp)
                        i3 = nc.scalar.activation(o, cc, AF.Tanh)
                        if dep_edges and last_sin is not None:
                            for bi in (i1, i2, i3):
                                add_dep_helper(bi.ins, last_sin, False,
                                               "act-set phase order")
                        prev_batch_last_tanh = i3.ins
                        nc.sync.dma_start(
                            out=y[r0:r1].rearrange("(p f) -> p f", p=P),
                            in_=o,
                        )
                        continue
                    def dtile(nm):
                        return tmp_pool.tile([P, F], F32, name=f"{nm}{t}",
                                             tag=nm, bufs=dve_bufs)
                    a = dtile("a")
                    bb = dtile("bb")
                    cc = tmp_pool.tile([P, F], F32, name=f"cc{t}", tag="cc")
                    e = tmp_pool.tile([P, F], F32, name=f"e{t}", tag="e")
                    f = dtile("f")
                    sq = tmp_pool.tile([P, F], F32, name=f"sq{t}", tag="sq")
                    e6 = tmp_pool.tile([P, F], F32, name=f"e6{t}", tag="e6")
                    d = dtile("d")
                    rc = None if fused_div else dtile("rc")
                    q = dtile("q")
                    r = dtile("r")
                    u = dtile("u")
                    o = tmp_pool.tile([P, F], ydt, name=f"o{t}", tag="o")

                    # GPSIMD: x5*x5 — the same-AP strided mult is cheap on
                    # Pool (~0.2us measured); copies there are NOT (~5us).
                    nc.gpsimd.tensor_tensor(
                        out=sq, in0=xv[:, :, 5], in1=xv[:, :, 5], op=OP.mult)

                    # ACT: cc=|x3| (Abs is in every table set), e=exp(-cc),
                    # e6=exp(x6)   (exp_and_others)
                    nc.scalar.activation(cc, xv[:, :, 3], AF.Abs)
                    i1 = nc.scalar.activation(e, cc, AF.Exp, scale=-1.0)
                    i2 = nc.scalar.activation(e6, xv[:, :, 6], AF.Exp)
                    if dep_edges and last_sin is not None:
                        for bi in (i1, i2):
                            add_dep_helper(bi.ins, last_sin, False,
                                           "act-set phase order")

                    # DVE chain (ops listed in gps_ops run on GPSIMD instead)
                    def eng_for(nm):
                        return nc.gpsimd if nm in gps_ops else nc.vector
                    eng_for("a").tensor_tensor(out=a, in0=xv[:, :, 0],
                                               in1=xv[:, :, 1],
                                               op=OP.mult)       # x0*x1
                    eng_for("bb").tensor_add(out=bb, in0=a, in1=st)
                    eng_for("f").tensor_tensor(out=f, in0=bb, in1=e,
                                               op=OP.mult)
                    eng_for("d").tensor_add(out=d, in0=sq, in1=e6)
                    if fused_div:
                        nc.vector._custom_dve(
                            DIV_OP, out=q, in0=d, in1=xv[:, :, 4],
                            s0=DIV_S0, s1=DIV_S1,
                        )                                        # q = x4/d
                    else:
                        nc.vector.reciprocal_approx_fast(out=rc, in_=d)
                        eng_for("q").tensor_tensor(out=q, in0=xv[:, :, 4],
                                                   in1=rc,
                                                   op=OP.mult)   # q = x4/d
                    eng_for("r").tensor_add(out=r, in0=f, in1=q)
                    eng_for("u").tensor_tensor(out=u, in0=r, in1=xv[:, :, 7],
                                               op=OP.subtract)

                    i3 = nc.scalar.activation(o, u, AF.Tanh)
                    if dep_edges and last_sin is not None:
                        add_dep_helper(i3.ins, last_sin, False,
                                       "act-set phase order")
                    prev_batch_last_tanh = i3.ins

                    nc.sync.dma_start(
                        out=y[r0:r1].rearrange("(p f) -> p f", p=P),
                        in_=o,
                    )
    nc.compile()
    return nc


_BUILT = None

# The configuration kernel() ships with (exp/sim drivers pass their own).
BUILD_KWARGS: dict = {
    "poly_sin": True, "fused_div": True, "bf16_out": True,
    "bf16_tmps": True, "sq_on_act": True, "gps_ops": (),
    "inp_bufs_over": 5, "tail_split": 2, "noskew": 0,
    "out_batches": (4, 4, 4, 2, 1),
}


def _get_built():
    global _BUILT
    if _BUILT is None:
        _BUILT = build_bass(**BUILD_KWARGS)
    return _BUILT


def run_spmd(inputs: np.ndarray, **kwargs) -> tuple[np.ndarray, object]:
    """Shard, run on 8 cores, gather.  Returns (full output, BassKernelResults).

    The axon-tunneled devices occasionally wedge transiently
    (NRT_EXEC_UNIT_UNRECOVERABLE); one retry after a pause usually
    recovers, so don't fail the whole run on the first error.
    """
    import time as _time
    x = np.ascontiguousarray(np.asarray(inputs, dtype=np.float32))
    assert x.shape == (N_ROWS, N_VARS), x.shape
    shards = x.reshape(N_CORES, ROWS_PER_CORE, N_VARS)
    in_maps = [{"x": np.ascontiguousarray(shards[i])} for i in range(N_CORES)]
    nc = _get_built()
    last_exc = None
    for attempt in range(3):
        try:
            res = bass_utils.run_bass_kernel_spmd(
                nc, in_maps, core_ids=list(range(N_CORES)), **kwargs
            )
            break
        except Exception as exc:  # transient device wedge — retry
            last_exc = exc
            _time.sleep(10 * (attempt + 1))
    else:
        raise last_exc
    out = np.concatenate([r["y"].reshape(-1) for r in res.results], axis=0)
    if out.dtype != np.float32:
        out = out.astype(np.float32)
    return out, res


def kernel(inputs: np.ndarray) -> np.ndarray:
    out, _ = run_spmd(inputs)
    return out

